# revision 1
# baseline (speedup 1.0000x reference)
"""MaxSimilarity (cosine-sim row-max) Trainium2 kernel.

out[i] = max_j  (x1[i] . x2[j]) / max(||x1[i]|| * ||x2[j]||, 1e-8)
x1: [8192, 1024] f32, x2: [16384, 1024] f32, out: [8192] f32.

Strategy (8 NeuronCores):
- Shard x2 rows 8-way (2048 rows/core); replicate x1. Each core computes the
  row-max over its j-shard for all 8192 queries; host combines shards with
  elementwise max.
- Rows of x1 and x2 are normalized to unit length on the host, so the device
  kernel is a pure matmul + row-max. Matmul runs in bf16: a single term
  gives ~1.3e-3 relative error on this data, far inside the 2e-2 gate.
  bf16 streams at 1 cycle/row like float32r, but (unlike float32r, whose
  4-byte weight path forces a full weight reload with every matmul) it gets
  Fast Weight Load, which shaves the per-matmul LDWEIGHTS overhead; it also
  halves the HBM traffic. 2048 matmuls x 512 cycles per core.
- Loop structure is j-block-outer over two resident panels of 32 query
  tiles: each pass over a panel needs only one 2 MB j-chunk of x2, so the
  PE starts after ~2.5 MB of DMA instead of waiting for the full 8 MB x2
  shard (which cost 35 us of dead PE time m-outer), and the ~68 us first
  pass gives the remaining chunks ample time to land (shorter first passes
  were measured to outrun the DMA and trip a HAM re-throttle).
- PSUM tiles [128 q, 512 j] are drained on DVE with a reduce-max over j into
  a per-(m,jb) column; after a panel's last pass each query tile's 4 block
  maxima are reduced and the result is written out once, contiguously, in
  [q_within_tile, m_tile] layout (the host untransposes -- a direct
  (m p)-ordered DMA scatters 8192 4-byte writes to HBM and costs ~25 us in
  write-completion latency).
"""

import ml_dtypes
import numpy as np

import concourse.bacc as bacc
import concourse.mybir as mybir
import concourse.tile as tile
from concourse.bass_utils import run_bass_kernel_spmd

N1, N2, D = 8192, 16384, 1024
P = 128
NCORES = 8
JS = N2 // NCORES          # 2048 j per core
JBLK = 512                 # psum moving free dim (one bank of fp32)
JB = JS // JBLK            # 4 psum blocks per core
M_TILES = N1 // P          # 64
K_TILES = D // P           # 8
MP = 32                    # m-tiles per panel (16 MB of x1 resident)
PARTS = M_TILES // MP      # 2 panels

F32 = mybir.dt.float32
BF16 = mybir.dt.bfloat16
ALU = mybir.AluOpType
AX = mybir.AxisListType


def build_nc():
    nc = bacc.Bacc(trn_type="TRN2")

    x1t = nc.dram_tensor("x1t", [M_TILES, P, K_TILES, P], BF16, kind="ExternalInput")
    x2t = nc.dram_tensor("x2t", [P, K_TILES, JS], BF16, kind="ExternalInput")
    out = nc.dram_tensor("out", [P, M_TILES], F32, kind="ExternalOutput")

    with tile.TileContext(nc) as tc:
        with (
            tc.tile_pool(name="resident", bufs=1) as res,
            tc.tile_pool(name="x1pool", bufs=MP) as x1pool,
            tc.tile_pool(name="psum", bufs=8, space="PSUM") as psum,
        ):
            # resident transposed x2 shard. dma_start issue costs ~650 ns
            # each (serial on the Sync engine), so use few, big DMAs and
            # issue them in consumption order, interleaved with the first
            # panel's x1 tiles: the first matmul group is gated on DMA #1
            # (x2 j-block 0) + DMA #2 (x1 tile 0) only.
            x2t_t = res.tile([P, K_TILES, JS], BF16, tag="x2t")
            cmax = res.tile([P, M_TILES, JB], F32, tag="cmax")
            rmax = res.tile([P, M_TILES], F32, tag="rmax")

            def load_x2_chunk(jb, ks=slice(0, K_TILES)):
                js = slice(jb * JBLK, (jb + 1) * JBLK)
                nc.sync.dma_start(out=x2t_t[:, ks, js], in_=x2t[:, ks, js])

            def load_x1(m):
                a = x1pool.tile([P, K_TILES, P], BF16, tag="x1")
                nc.sync.dma_start(out=a[:], in_=x1t[m])
                return a

            # (m_start, m_count, order). Two 32-tile panels, both
            # j-block-outer: the first pass over 32 query tiles runs ~68 us
            # off x2 chunk 0 alone, which is ample time for chunks 1-3 to
            # land (an 8-tile first pass was measured to outrun the DMA and
            # trip a HAM re-throttle). j-block-outer also interleaves the
            # PSUM drains with matmuls, so nothing bunches after the last MM.
            parts = [(0, 32, "jb"), (32, 32, "jb")]

            def jb_outer(tiles, m0, cnt, skip=0):
                for jb in range(JB):
                    js = slice(jb * JBLK, (jb + 1) * JBLK)
                    for mi in range(cnt):
                        if jb == 0 and mi < skip:
                            continue
                        m = m0 + mi
                        ps = psum.tile([P, JBLK], F32, tag="ps")
                        for k in range(K_TILES):
                            nc.tensor.matmul(
                                ps[:], tiles[mi][:, k, :], x2t_t[:, k, js],
                                start=(k == 0), stop=(k == K_TILES - 1),
                            )
                        nc.vector.tensor_reduce(
                            cmax[:, m, jb : jb + 1], ps[:], axis=AX.X, op=ALU.max
                        )
                        if jb == JB - 1:
                            nc.vector.tensor_reduce(
                                rmax[:, m : m + 1], cmax[:, m, :], axis=AX.X, op=ALU.max
                            )
                            if (mi + 1) % 8 == 0:
                                nc.sync.dma_start(
                                    out=out[:, m - 7 : m + 1],
                                    in_=rmax[:, m - 7 : m + 1],
                                )

            def k_outer(tiles, m0, cnt, skip=0):
                for mi in range(cnt):
                    m = m0 + mi
                    pss = [psum.tile([P, JBLK], F32, tag="ps", name="ps") for _ in range(JB)]
                    for k in range(K_TILES):
                        for jb in range(JB):
                            js = slice(jb * JBLK, (jb + 1) * JBLK)
                            nc.tensor.matmul(
                                pss[jb][:], tiles[mi][:, k, :], x2t_t[:, k, js],
                                start=(k == 0), stop=(k == K_TILES - 1),
                            )
                    for jb in range(JB):
                        nc.vector.tensor_reduce(
                            cmax[:, m, jb : jb + 1], pss[jb][:], axis=AX.X, op=ALU.max
                        )
                    nc.vector.tensor_reduce(
                        rmax[:, m : m + 1], cmax[:, m, :], axis=AX.X, op=ALU.max
                    )

            # PE warm-up: matmuls on memset zeros, no DMA dependency.
            # They run during the initial DMA wait, flip the HAM clock gate
            # to 8/8, and finish about when the first real operands land --
            # so the real stream starts at full rate instead of paying the
            # ~3.4 us half-speed ramp.
            warm_a = res.tile([P, P], BF16, tag="warma")
            warm_b = res.tile([P, JBLK], BF16, tag="warmb")
            nc.any.memset(warm_a[:], 0)
            nc.any.memset(warm_b[:], 0)
            wps = psum.tile([P, JBLK], F32, tag="ps")
            for _ in range(12):
                nc.tensor.matmul(wps[:], warm_a[:], warm_b[:], start=True, stop=True)

            KH = K_TILES // 2
            for pi, (m0, cnt, order) in enumerate(parts):
                if pi == 0:
                    # x1 tile 0 first, then the k 0..3 slices of chunk 0
                    # (the proven v3 DMA shape) -- together they gate the
                    # opening groups, so issue exactly them before anything
                    tiles = [load_x1(m0)]
                    for k in range(KH):
                        nc.sync.dma_start(
                            out=x2t_t[:, k, 0:JBLK], in_=x2t[:, k, 0:JBLK]
                        )
                    tiles += [load_x1(m0 + mi) for mi in range(1, 4)]
                    for k in range(KH, K_TILES):
                        nc.sync.dma_start(
                            out=x2t_t[:, k, 0:JBLK], in_=x2t[:, k, 0:JBLK]
                        )
                    tiles += [load_x1(m0 + mi) for mi in range(4, cnt)]
                    # m0..m3, j-block 0: accumulate k 0..3 while the second
                    # k-half of the chunk is still in flight
                    open_ps = []
                    for mi in range(4):
                        ps = psum.tile([P, JBLK], F32, tag="ps")
                        for k in range(KH):
                            nc.tensor.matmul(
                                ps[:], tiles[mi][:, k, :], x2t_t[:, k, 0:JBLK],
                                start=(k == 0), stop=False,
                            )
                        open_ps.append(ps)
                    for mi in range(4):
                        ps = open_ps[mi]
                        for k in range(KH, K_TILES):
                            nc.tensor.matmul(
                                ps[:], tiles[mi][:, k, :], x2t_t[:, k, 0:JBLK],
                                start=False, stop=(k == K_TILES - 1),
                            )
                        nc.vector.tensor_reduce(
                            cmax[:, m0 + mi, 0:1], ps[:], axis=AX.X, op=ALU.max
                        )
                    for jb in range(1, JB):
                        load_x2_chunk(jb)
                else:
                    tiles = [load_x1(m0 + mi) for mi in range(cnt)]
                (jb_outer if order == "jb" else k_outer)(
                    tiles, m0, cnt, skip=4 if pi == 0 else 0
                )

    nc.finalize()
    return nc


_cache = {}


def _get_nc():
    if "nc" not in _cache:
        _cache["nc"] = build_nc()
    return _cache["nc"]


def _prep_inputs(x1, x2):
    """Host-side prep: row-normalize, TF32-round, transpose + tile + shard."""
    x1 = np.ascontiguousarray(x1, dtype=np.float32)
    x2 = np.ascontiguousarray(x2, dtype=np.float32)
    eps = np.float32(1e-8)
    n1 = np.maximum(np.sqrt(np.einsum("ij,ij->i", x1, x1)), eps)
    n2 = np.maximum(np.sqrt(np.einsum("ij,ij->i", x2, x2)), eps)
    x1 = (x1 / n1[:, None]).astype(ml_dtypes.bfloat16)
    x2 = (x2 / n2[:, None]).astype(ml_dtypes.bfloat16)

    # [N1, D] -> [m, dp, k, q] with x1t[m, dp, k, q] = x1[m*128+q, k*128+dp]
    x1t = np.ascontiguousarray(
        x1.reshape(M_TILES, P, K_TILES, P).transpose(0, 3, 2, 1)
    )

    in_maps = []
    for c in range(NCORES):
        sl = slice(c * JS, (c + 1) * JS)
        # [JS, D] -> [dp, k, j] with x2t[dp, k, j] = x2[sl][j, k*128+dp]
        x2t = np.ascontiguousarray(
            x2[sl].T.reshape(K_TILES, P, JS).transpose(1, 0, 2)
        )
        in_maps.append({"x1t": x1t, "x2t": x2t})
    return in_maps


def run(x1, x2, trace=False):
    nc = _get_nc()
    in_maps = _prep_inputs(x1, x2)
    res = run_bass_kernel_spmd(nc, in_maps, core_ids=list(range(NCORES)), trace=trace)
    # device output is [q_within_tile, m_tile]; out[m*128+q] = arr[q, m]
    parts = [np.asarray(res.results[c]["out"]).reshape(P, M_TILES) for c in range(NCORES)]
    out = np.maximum.reduce(parts).T.ravel().astype(np.float32)
    return np.ascontiguousarray(out), res


def kernel(x1, x2):
    out, _ = run(np.asarray(x1), np.asarray(x2), trace=False)
    return out



# revision 3
# speedup vs baseline: 1.0066x; 1.0066x over previous
"""MaxSimilarity via fp8-e4m3 DoubleRow screen + bf16 rescore.

out[i] = max_j (x1[i].x2[j]) / max(||x1[i]||*||x2[j]||, 1e-8)
x1: [8192,1024] f32, x2: [16384,1024] f32, out: [8192] f32.

Two phases:
  L1 (device): row-normalized inputs quantized to e4m3 (scaled by 64 to
     stay out of the subnormal range); GEMM with perf_mode=DoubleRow
     (2 fp8 weights/PE cell, 256-deep contraction per matmul) computes
     all 16384 sims/query; DVE reduces each PSUM tile to per-32-column
     block maxes. Output: [128, 64, 64] f32 block maxes per core.
  L2 (host for now): global per-query max over the 256*... blocks;
     every block within DELTA of the max is rescored in bf16; the final
     value comes only from rescored blocks, so accuracy is bf16-level
     (~1.3e-3 rel) while the screen only has to rank blocks to within
     DELTA (fp8 dot noise sigma ~1e-3 abs).

x2 is sharded 8 ways over j (2048/core); x1 replicated. All operands
fp8-resident in SBUF (x1 8MB + x2 2MB), so no paneling: per m-tile,
k-pair-outer / j-block-inner order gives each DoubleRow weight tile a
4x512-column stream to hide its (FWL-less) 256-column LDWEIGHTS.
"""

import ml_dtypes
import numpy as np

import concourse.bacc as bacc
import concourse.mybir as mybir
import concourse.tile as tile
from concourse.bass_utils import run_bass_kernel_spmd

N1, N2, D = 8192, 16384, 1024
P = 128
NCORES = 8
JS = N2 // NCORES          # 2048 j per core
JBLK = 512                 # psum moving free dim (one bank of fp32)
JB = JS // JBLK            # 4 psum blocks per core
M_TILES = N1 // P          # 64
K_TILES = D // P           # 8
KP = K_TILES // 2          # 4 k-pairs (256-deep DoubleRow contraction)
BW = 32                    # screen block width (j per block max)
NBLK = JS // BW            # 64 blocks per core
BPP = JBLK // BW           # 16 blocks per psum tile
HEAD_M = 4                 # m-tiles in the DMA-overlapped head panel

SCALE = np.float32(64.0)   # fp8 pre-scale; sims come back *SCALE^2
DELTA = 8e-3               # rescore margin in sim units

F32 = mybir.dt.float32
FP8 = mybir.dt.float8e4
ALU = mybir.AluOpType
AX = mybir.AxisListType
DR = mybir.MatmulPerfMode.DoubleRow


def build_nc():
    nc = bacc.Bacc(trn_type="TRN2")

    x1t = nc.dram_tensor("x1t", [M_TILES, P, K_TILES, P], FP8, kind="ExternalInput")
    x2t = nc.dram_tensor("x2t", [P, K_TILES, JS], FP8, kind="ExternalInput")
    out = nc.dram_tensor("out", [P, M_TILES, NBLK], F32, kind="ExternalOutput")

    with tile.TileContext(nc) as tc:
        with (
            tc.tile_pool(name="resident", bufs=1) as res,
            tc.tile_pool(name="psum", bufs=8, space="PSUM") as psum,
        ):
            x2t_s = res.tile([P, K_TILES, JS], FP8, tag="x2t")
            x1t_s = res.tile([P, M_TILES, K_TILES, P], FP8, tag="x1t")
            cmax = res.tile([P, M_TILES, NBLK], F32, tag="cmax")

            # PE warm-up on memset zeros: no DMA dependency; its job is to
            # keep the PE continuously busy >=3.24us so the HAM clock gate
            # flips to 8/8 during warm-up (a PE gap before the flip delays
            # it; a >1us gap after it re-throttles). memsets go through DVE
            # explicitly -- nc.any routes big ones to the scalar engine,
            # whose ACT_TABLE_LOAD init adds ~2.8us serial latency. warm_b
            # is memset in halves so the first (256-col) warm-up matmul
            # starts after 256B/partition.
            warm_a = res.tile([P, 2, P], FP8, tag="warma")
            warm_b = res.tile([P, 2, JBLK], FP8, tag="warmb")
            nc.vector.memset(warm_a[:], 0)
            nc.vector.memset(warm_b[:, :, 0:JBLK // 2], 0)
            nc.vector.memset(warm_b[:, :, JBLK // 2:], 0)
            wps = psum.tile([P, JBLK], F32, tag="ps")
            for _ in range(2):
                nc.tensor.matmul(wps[:, 0:JBLK // 2], warm_a[:],
                                 warm_b[:, :, 0:JBLK // 2],
                                 start=True, stop=True, perf_mode=DR)
            for _ in range(8):
                nc.tensor.matmul(wps[:], warm_a[:], warm_b[:],
                                 start=True, stop=True, perf_mode=DR)

            # DMAs in consumption order. Measured: no packet lands before
            # ~8.7us and the 2MB x2 shard completes only at ~17us
            # (150-380GB/s ramp), so the head must consume x2 at below
            # delivery rate: x1 tiles 0-3, then the 16 [k-pair, j-block]
            # 128KB pieces of x2 in the j-major order the head panel
            # consumes them, then the rest of x1 batched 8 tiles per DMA
            # (transposed source AP puts the partition dim first to match
            # the SBUF iteration order).
            for m in range(HEAD_M):
                nc.sync.dma_start(out=x1t_s[:, m], in_=x1t[m])
            for jb in range(JB):
                js = slice(jb * JBLK, (jb + 1) * JBLK)
                for t in range(KP):
                    ks = slice(2 * t, 2 * t + 2)
                    nc.sync.dma_start(out=x2t_s[:, ks, js], in_=x2t[:, ks, js])
            for g in range(HEAD_M, M_TILES, 8):
                hi = min(g + 8, M_TILES)
                nc.sync.dma_start(out=x1t_s[:, g:hi],
                                  in_=x1t[g:hi].transpose([1, 0, 2, 3]))

            # Head panel: m-tiles 0-3 j-block-outer, one PSUM bank per
            # m-tile per j-block. A single m-tile would read x2 at
            # ~590GB/s and starve (run7: 2.4us gap -> HAM re-throttle,
            # +2us); four m-tiles per piece consume at ~148GB/s, under
            # the measured delivery ramp, so the head runs gap-free while
            # x2 streams in.
            for jb in range(JB):
                js = slice(jb * JBLK, (jb + 1) * JBLK)
                pssp = [psum.tile([P, JBLK], F32, tag="ps", name="ps")
                        for _ in range(HEAD_M)]
                for mi in range(HEAD_M):
                    for t in range(KP):
                        ks = slice(2 * t, 2 * t + 2)
                        nc.tensor.matmul(
                            pssp[mi][:], x1t_s[:, mi, ks, :], x2t_s[:, ks, js],
                            start=(t == 0), stop=(t == KP - 1), perf_mode=DR,
                        )
                    nc.vector.tensor_reduce(
                        cmax[:, mi, jb * BPP:(jb + 1) * BPP],
                        pssp[mi][:].rearrange("p (b w) -> p b w", b=BPP),
                        axis=AX.X, op=ALU.max,
                    )

            for m in range(HEAD_M, M_TILES):
                pss = [psum.tile([P, JBLK], F32, tag="ps", name="ps")
                       for _ in range(JB)]
                for t in range(KP):
                    ks = slice(2 * t, 2 * t + 2)
                    for jb in range(JB):
                        js = slice(jb * JBLK, (jb + 1) * JBLK)
                        nc.tensor.matmul(
                            pss[jb][:], x1t_s[:, m, ks, :], x2t_s[:, ks, js],
                            start=(t == 0), stop=(t == KP - 1), perf_mode=DR,
                        )
                for jb in range(JB):
                    nc.vector.tensor_reduce(
                        cmax[:, m, jb * BPP:(jb + 1) * BPP],
                        pss[jb][:].rearrange("p (b w) -> p b w", b=BPP),
                        axis=AX.X, op=ALU.max,
                    )
                # drain to HBM every 8 m-tiles; finer cadence at the end so
                # the post-last-matmul DMA is small.
                if m < 56 and (m + 1) % 8 == 0:
                    nc.sync.dma_start(
                        out=out[:, m - 7:m + 1, :], in_=cmax[:, m - 7:m + 1, :]
                    )
                elif m >= 56 and (m + 1) % 2 == 0:
                    nc.sync.dma_start(
                        out=out[:, m - 1:m + 1, :], in_=cmax[:, m - 1:m + 1, :]
                    )

    nc.finalize()
    return nc


_cache = {}


def _get_nc():
    if "nc" not in _cache:
        _cache["nc"] = build_nc()
    return _cache["nc"]


def _normalize(x1, x2):
    x1 = np.ascontiguousarray(x1, dtype=np.float32)
    x2 = np.ascontiguousarray(x2, dtype=np.float32)
    eps = np.float32(1e-8)
    n1 = np.maximum(np.sqrt(np.einsum("ij,ij->i", x1, x1)), eps)
    n2 = np.maximum(np.sqrt(np.einsum("ij,ij->i", x2, x2)), eps)
    return x1 / n1[:, None], x2 / n2[:, None]


def _prep_inputs(x1n, x2n):
    """fp8-quantize (scaled), transpose + tile + shard."""
    q1 = (x1n * SCALE).astype(ml_dtypes.float8_e4m3)
    q2 = (x2n * SCALE).astype(ml_dtypes.float8_e4m3)

    # [N1, D] -> [m, dp, k, q] with x1t[m, dp, k, q] = q1[m*128+q, k*128+dp]
    x1t = np.ascontiguousarray(
        q1.reshape(M_TILES, P, K_TILES, P).transpose(0, 3, 2, 1)
    )
    in_maps = []
    for c in range(NCORES):
        sl = slice(c * JS, (c + 1) * JS)
        # [JS, D] -> [dp, k, j] with x2t[dp, k, j] = q2[sl][j, k*128+dp]
        x2t = np.ascontiguousarray(
            q2[sl].T.reshape(K_TILES, P, JS).transpose(1, 0, 2)
        )
        in_maps.append({"x1t": x1t, "x2t": x2t})
    return in_maps


def _host_rescore(bm, x1n, x2n):
    """bm: [8192, NCORES*NBLK] screen block maxes (sim units).
    Rescore every block within DELTA of each query's max in bf16."""
    b16 = lambda a: np.ascontiguousarray(a.astype(ml_dtypes.bfloat16)
                                          .astype(np.float32))
    x1b = b16(x1n)
    x2b = b16(x2n)
    gmax = bm.max(axis=1, keepdims=True)
    sel = bm >= gmax - np.float32(DELTA)
    out = np.full(N1, -np.inf, dtype=np.float32)
    nb_total = NCORES * NBLK
    for b in range(nb_total):
        qs = np.nonzero(sel[:, b])[0]
        if len(qs) == 0:
            continue
        blk = x1b[qs] @ x2b[b * BW:(b + 1) * BW].T
        np.maximum.at(out, qs, blk.max(axis=1))
    return out


def run(x1, x2, trace=False):
    nc = _get_nc()
    x1n, x2n = _normalize(np.asarray(x1), np.asarray(x2))
    in_maps = _prep_inputs(x1n, x2n)
    res = run_bass_kernel_spmd(nc, in_maps, core_ids=list(range(NCORES)),
                               trace=trace)
    # device out[q, m, b] = screen blockmax of query m*128+q, local block b
    bm = np.concatenate(
        [np.asarray(res.results[c]["out"]).reshape(P, M_TILES, NBLK)
         .transpose(1, 0, 2).reshape(N1, NBLK) for c in range(NCORES)],
        axis=1,
    ) / (SCALE * SCALE)
    out = _host_rescore(bm, x1n, x2n)
    return np.ascontiguousarray(out.astype(np.float32)), res


def kernel(x1, x2):
    out, _ = run(np.asarray(x1), np.asarray(x2), trace=False)
    return out


# revision 4
# speedup vs baseline: 1.0095x; 1.0029x over previous
"""MaxSimilarity via fp8-e4m3 DoubleRow screen + bf16 rescore.

out[i] = max_j (x1[i].x2[j]) / max(||x1[i]||*||x2[j]||, 1e-8)
x1: [8192,1024] f32, x2: [16384,1024] f32, out: [8192] f32.

Two phases:
  L1 (device): row-normalized inputs quantized to e4m3 (scaled by 64 to
     stay out of the subnormal range); GEMM with perf_mode=DoubleRow
     (2 fp8 weights/PE cell, 256-deep contraction per matmul) computes
     all 16384 sims/query; DVE reduces each PSUM tile to per-32-column
     block maxes. Output: [128, 64, 64] f32 block maxes per core.
  L2 (host for now): global per-query max over the 256*... blocks;
     every block within DELTA of the max is rescored in bf16; the final
     value comes only from rescored blocks, so accuracy is bf16-level
     (~1.3e-3 rel) while the screen only has to rank blocks to within
     DELTA (fp8 dot noise sigma ~1e-3 abs).

x2 is sharded 8 ways over j (2048/core); x1 replicated. All operands
fp8-resident in SBUF (x1 8MB + x2 2MB), so no paneling: per m-tile,
k-pair-outer / j-block-inner order gives each DoubleRow weight tile a
4x512-column stream to hide its (FWL-less) 256-column LDWEIGHTS.
"""

import ml_dtypes
import numpy as np

import concourse.bacc as bacc
import concourse.mybir as mybir
import concourse.tile as tile
from concourse.bass_utils import run_bass_kernel_spmd

N1, N2, D = 8192, 16384, 1024
P = 128
NCORES = 8
JS = N2 // NCORES          # 2048 j per core
JBLK = 512                 # psum moving free dim (one bank of fp32)
JB = JS // JBLK            # 4 psum blocks per core
M_TILES = N1 // P          # 64
K_TILES = D // P           # 8
KP = K_TILES // 2          # 4 k-pairs (256-deep DoubleRow contraction)
BW = 32                    # screen block width (j per block max)
NBLK = JS // BW            # 64 blocks per core
BPP = JBLK // BW           # 16 blocks per psum tile
HEAD_M = 4                 # m-tiles in the DMA-overlapped head panel

SCALE = np.float32(64.0)   # fp8 pre-scale; sims come back *SCALE^2
DELTA = 8e-3               # rescore margin in sim units

F32 = mybir.dt.float32
FP8 = mybir.dt.float8e4
ALU = mybir.AluOpType
AX = mybir.AxisListType
DR = mybir.MatmulPerfMode.DoubleRow


def build_nc():
    nc = bacc.Bacc(trn_type="TRN2")

    x1t = nc.dram_tensor("x1t", [M_TILES, P, K_TILES, P], FP8, kind="ExternalInput")
    x2t = nc.dram_tensor("x2t", [P, K_TILES, JS], FP8, kind="ExternalInput")
    out = nc.dram_tensor("out", [P, M_TILES, NBLK], F32, kind="ExternalOutput")

    with tile.TileContext(nc) as tc:
        with (
            tc.tile_pool(name="resident", bufs=1) as res,
            tc.tile_pool(name="psum", bufs=8, space="PSUM") as psum,
        ):
            x2t_s = res.tile([P, K_TILES, JS], FP8, tag="x2t")
            x1t_s = res.tile([P, M_TILES, K_TILES, P], FP8, tag="x1t")
            cmax = res.tile([P, M_TILES, NBLK], F32, tag="cmax")

            # PE warm-up on memset zeros: no DMA dependency; its job is to
            # keep the PE continuously busy >=3.24us so the HAM clock gate
            # flips to 8/8 during warm-up (a PE gap before the flip delays
            # it; a >1us gap after it re-throttles). memsets go through DVE
            # explicitly -- nc.any routes big ones to the scalar engine,
            # whose ACT_TABLE_LOAD init adds ~2.8us serial latency. warm_b
            # is memset in halves so the first (256-col) warm-up matmul
            # starts after 256B/partition.
            warm_a = res.tile([P, 2, P], FP8, tag="warma")
            warm_b = res.tile([P, 2, JBLK], FP8, tag="warmb")
            nc.vector.memset(warm_a[:], 0)
            nc.vector.memset(warm_b[:, :, 0:JBLK // 2], 0)
            nc.vector.memset(warm_b[:, :, JBLK // 2:], 0)
            wps = psum.tile([P, JBLK], F32, tag="ps")
            for _ in range(2):
                nc.tensor.matmul(wps[:, 0:JBLK // 2], warm_a[:],
                                 warm_b[:, :, 0:JBLK // 2],
                                 start=True, stop=True, perf_mode=DR)
            for _ in range(8):
                nc.tensor.matmul(wps[:], warm_a[:], warm_b[:],
                                 start=True, stop=True, perf_mode=DR)

            # DMAs in consumption order. Measured: no packet lands before
            # ~8.7us and the 2MB x2 shard completes only at ~17us
            # (150-380GB/s ramp), so the head must consume x2 at below
            # delivery rate: x1 tiles 0-3, then the 16 [k-pair, j-block]
            # 128KB pieces of x2 in the j-major order the head panel
            # consumes them, then the rest of x1 batched 8 tiles per DMA
            # (transposed source AP puts the partition dim first to match
            # the SBUF iteration order).
            # interleave the head x1 tiles with j-block 0's pieces in exact
            # consumption order: mi=0 needs x1t[0]+k0j0 first; x1t[1..3]
            # queued ahead of x2 delayed k0j0 past the warm-up drain and
            # cost a 2.2us gap + HAM re-throttle.
            nc.sync.dma_start(out=x1t_s[:, 0], in_=x1t[0])
            nc.sync.dma_start(out=x2t_s[:, 0:2, 0:JBLK],
                              in_=x2t[:, 0:2, 0:JBLK])
            for t in range(1, KP):
                ks = slice(2 * t, 2 * t + 2)
                nc.sync.dma_start(out=x2t_s[:, ks, 0:JBLK],
                                  in_=x2t[:, ks, 0:JBLK])
                nc.sync.dma_start(out=x1t_s[:, t], in_=x1t[t])
            for jb in range(1, JB):
                js = slice(jb * JBLK, (jb + 1) * JBLK)
                for t in range(KP):
                    ks = slice(2 * t, 2 * t + 2)
                    nc.sync.dma_start(out=x2t_s[:, ks, js], in_=x2t[:, ks, js])
            for g in range(HEAD_M, M_TILES, 8):
                hi = min(g + 8, M_TILES)
                nc.sync.dma_start(out=x1t_s[:, g:hi],
                                  in_=x1t[g:hi].transpose([1, 0, 2, 3]))

            # Head panel: m-tiles 0-3 j-block-outer, one PSUM bank per
            # m-tile per j-block. A single m-tile would read x2 at
            # ~590GB/s and starve (run7: 2.4us gap -> HAM re-throttle,
            # +2us); four m-tiles per piece consume at ~148GB/s, under
            # the measured delivery ramp, so the head runs gap-free while
            # x2 streams in.
            for jb in range(JB):
                js = slice(jb * JBLK, (jb + 1) * JBLK)
                pssp = [psum.tile([P, JBLK], F32, tag="ps", name="ps")
                        for _ in range(HEAD_M)]
                for mi in range(HEAD_M):
                    for t in range(KP):
                        ks = slice(2 * t, 2 * t + 2)
                        nc.tensor.matmul(
                            pssp[mi][:], x1t_s[:, mi, ks, :], x2t_s[:, ks, js],
                            start=(t == 0), stop=(t == KP - 1), perf_mode=DR,
                        )
                    nc.vector.tensor_reduce(
                        cmax[:, mi, jb * BPP:(jb + 1) * BPP],
                        pssp[mi][:].rearrange("p (b w) -> p b w", b=BPP),
                        axis=AX.X, op=ALU.max,
                    )

            for m in range(HEAD_M, M_TILES):
                pss = [psum.tile([P, JBLK], F32, tag="ps", name="ps")
                       for _ in range(JB)]
                for t in range(KP):
                    ks = slice(2 * t, 2 * t + 2)
                    for jb in range(JB):
                        js = slice(jb * JBLK, (jb + 1) * JBLK)
                        nc.tensor.matmul(
                            pss[jb][:], x1t_s[:, m, ks, :], x2t_s[:, ks, js],
                            start=(t == 0), stop=(t == KP - 1), perf_mode=DR,
                        )
                for jb in range(JB):
                    nc.vector.tensor_reduce(
                        cmax[:, m, jb * BPP:(jb + 1) * BPP],
                        pss[jb][:].rearrange("p (b w) -> p b w", b=BPP),
                        axis=AX.X, op=ALU.max,
                    )
                # drain to HBM every 8 m-tiles; finer cadence at the end so
                # the post-last-matmul DMA is small.
                if m < 56 and (m + 1) % 8 == 0:
                    nc.sync.dma_start(
                        out=out[:, m - 7:m + 1, :], in_=cmax[:, m - 7:m + 1, :]
                    )
                elif m >= 56 and (m + 1) % 2 == 0:
                    nc.sync.dma_start(
                        out=out[:, m - 1:m + 1, :], in_=cmax[:, m - 1:m + 1, :]
                    )

    nc.finalize()
    return nc


_cache = {}


def _get_nc():
    if "nc" not in _cache:
        _cache["nc"] = build_nc()
    return _cache["nc"]


def _normalize(x1, x2):
    x1 = np.ascontiguousarray(x1, dtype=np.float32)
    x2 = np.ascontiguousarray(x2, dtype=np.float32)
    eps = np.float32(1e-8)
    n1 = np.maximum(np.sqrt(np.einsum("ij,ij->i", x1, x1)), eps)
    n2 = np.maximum(np.sqrt(np.einsum("ij,ij->i", x2, x2)), eps)
    return x1 / n1[:, None], x2 / n2[:, None]


def _prep_inputs(x1n, x2n):
    """fp8-quantize (scaled), transpose + tile + shard."""
    q1 = (x1n * SCALE).astype(ml_dtypes.float8_e4m3)
    q2 = (x2n * SCALE).astype(ml_dtypes.float8_e4m3)

    # [N1, D] -> [m, dp, k, q] with x1t[m, dp, k, q] = q1[m*128+q, k*128+dp]
    x1t = np.ascontiguousarray(
        q1.reshape(M_TILES, P, K_TILES, P).transpose(0, 3, 2, 1)
    )
    in_maps = []
    for c in range(NCORES):
        sl = slice(c * JS, (c + 1) * JS)
        # [JS, D] -> [dp, k, j] with x2t[dp, k, j] = q2[sl][j, k*128+dp]
        x2t = np.ascontiguousarray(
            q2[sl].T.reshape(K_TILES, P, JS).transpose(1, 0, 2)
        )
        in_maps.append({"x1t": x1t, "x2t": x2t})
    return in_maps


def _host_rescore(bm, x1n, x2n):
    """bm: [8192, NCORES*NBLK] screen block maxes (sim units).
    Rescore every block within DELTA of each query's max in bf16."""
    b16 = lambda a: np.ascontiguousarray(a.astype(ml_dtypes.bfloat16)
                                          .astype(np.float32))
    x1b = b16(x1n)
    x2b = b16(x2n)
    gmax = bm.max(axis=1, keepdims=True)
    sel = bm >= gmax - np.float32(DELTA)
    out = np.full(N1, -np.inf, dtype=np.float32)
    nb_total = NCORES * NBLK
    for b in range(nb_total):
        qs = np.nonzero(sel[:, b])[0]
        if len(qs) == 0:
            continue
        blk = x1b[qs] @ x2b[b * BW:(b + 1) * BW].T
        np.maximum.at(out, qs, blk.max(axis=1))
    return out


def run(x1, x2, trace=False):
    nc = _get_nc()
    x1n, x2n = _normalize(np.asarray(x1), np.asarray(x2))
    in_maps = _prep_inputs(x1n, x2n)
    res = run_bass_kernel_spmd(nc, in_maps, core_ids=list(range(NCORES)),
                               trace=trace)
    # device out[q, m, b] = screen blockmax of query m*128+q, local block b
    bm = np.concatenate(
        [np.asarray(res.results[c]["out"]).reshape(P, M_TILES, NBLK)
         .transpose(1, 0, 2).reshape(N1, NBLK) for c in range(NCORES)],
        axis=1,
    ) / (SCALE * SCALE)
    out = _host_rescore(bm, x1n, x2n)
    return np.ascontiguousarray(out.astype(np.float32)), res


def kernel(x1, x2):
    out, _ = run(np.asarray(x1), np.asarray(x2), trace=False)
    return out


# revision 5
# speedup vs baseline: 1.0115x; 1.0020x over previous
"""MaxSimilarity via fp8-e4m3 DoubleRow screen + bf16 rescore.

out[i] = max_j (x1[i].x2[j]) / max(||x1[i]||*||x2[j]||, 1e-8)
x1: [8192,1024] f32, x2: [16384,1024] f32, out: [8192] f32.

Two phases:
  L1 (device): row-normalized inputs quantized to e4m3 (scaled by 64 to
     stay out of the subnormal range); GEMM with perf_mode=DoubleRow
     (2 fp8 weights/PE cell, 256-deep contraction per matmul) computes
     all 16384 sims/query; DVE reduces each PSUM tile to per-32-column
     block maxes. Output: [128, 64, 64] f32 block maxes per core.
  L2 (host for now): global per-query max over the 256*... blocks;
     every block within DELTA of the max is rescored in bf16; the final
     value comes only from rescored blocks, so accuracy is bf16-level
     (~1.3e-3 rel) while the screen only has to rank blocks to within
     DELTA (fp8 dot noise sigma ~1e-3 abs).

x2 is sharded 8 ways over j (2048/core); x1 replicated. All operands
fp8-resident in SBUF (x1 8MB + x2 2MB), so no paneling: per m-tile,
k-pair-outer / j-block-inner order gives each DoubleRow weight tile a
4x512-column stream to hide its (FWL-less) 256-column LDWEIGHTS.
"""

import ml_dtypes
import numpy as np

import concourse.bacc as bacc
import concourse.mybir as mybir
import concourse.tile as tile
from concourse.bass_utils import run_bass_kernel_spmd

N1, N2, D = 8192, 16384, 1024
P = 128
NCORES = 8
JS = N2 // NCORES          # 2048 j per core
JBLK = 512                 # psum moving free dim (one bank of fp32)
JB = JS // JBLK            # 4 psum blocks per core
M_TILES = N1 // P          # 64
K_TILES = D // P           # 8
KP = K_TILES // 2          # 4 k-pairs (256-deep DoubleRow contraction)
BW = 32                    # screen block width (j per block max)
NBLK = JS // BW            # 64 blocks per core
BPP = JBLK // BW           # 16 blocks per psum tile
HEAD_M = 4                 # m-tiles in the DMA-overlapped head panel

SCALE = np.float32(64.0)   # fp8 pre-scale; sims come back *SCALE^2
DELTA = 8e-3               # rescore margin in sim units

F32 = mybir.dt.float32
FP8 = mybir.dt.float8e4
ALU = mybir.AluOpType
AX = mybir.AxisListType
DR = mybir.MatmulPerfMode.DoubleRow


def build_nc():
    nc = bacc.Bacc(trn_type="TRN2")

    x1t = nc.dram_tensor("x1t", [M_TILES, P, K_TILES, P], FP8, kind="ExternalInput")
    x2t = nc.dram_tensor("x2t", [P, K_TILES, JS], FP8, kind="ExternalInput")
    out = nc.dram_tensor("out", [P, M_TILES, NBLK], F32, kind="ExternalOutput")

    with tile.TileContext(nc) as tc:
        with (
            tc.tile_pool(name="resident", bufs=1) as res,
            tc.tile_pool(name="psum", bufs=8, space="PSUM") as psum,
        ):
            x2t_s = res.tile([P, K_TILES, JS], FP8, tag="x2t")
            x1t_s = res.tile([P, M_TILES, K_TILES, P], FP8, tag="x1t")
            cmax = res.tile([P, M_TILES, NBLK], F32, tag="cmax")

            # PE warm-up on memset zeros: no DMA dependency; its job is to
            # keep the PE continuously busy >=3.24us so the HAM clock gate
            # flips to 8/8 during warm-up (a PE gap before the flip delays
            # it; a >1us gap after it re-throttles). memsets go through DVE
            # explicitly -- nc.any routes big ones to the scalar engine,
            # whose ACT_TABLE_LOAD init adds ~2.8us serial latency. warm_b
            # is memset in halves so the first (256-col) warm-up matmul
            # starts after 256B/partition.
            warm_a = res.tile([P, 2, P], FP8, tag="warma")
            warm_b = res.tile([P, 2, JBLK], FP8, tag="warmb")
            nc.vector.memset(warm_a[:], 0)
            nc.vector.memset(warm_b[:, :, 0:JBLK // 2], 0)
            nc.vector.memset(warm_b[:, :, JBLK // 2:], 0)
            # bootstrap: the very first matmuls use warm_a for BOTH
            # operands (256B/partition memset, ~270ns) so the PE starts
            # ~1us before warm_b's halves are even set.
            wps = psum.tile([P, JBLK], F32, tag="ps")
            for _ in range(2):
                nc.tensor.matmul(wps[:, 0:P], warm_a[:], warm_a[:],
                                 start=True, stop=True, perf_mode=DR)
            for _ in range(2):
                nc.tensor.matmul(wps[:, 0:JBLK // 2], warm_a[:],
                                 warm_b[:, :, 0:JBLK // 2],
                                 start=True, stop=True, perf_mode=DR)
            # 9 full warm-ups: drain at ~11.9us, when the first x2 pieces
            # have actually landed (DMA physics is fixed; draining earlier
            # just exposes a pre-flip gap that delays the HAM flip). The
            # flip itself is locked at first-MM+3.24 ~= 11.0 by the early
            # bootstrap above.
            for _ in range(9):
                nc.tensor.matmul(wps[:], warm_a[:], warm_b[:],
                                 start=True, stop=True, perf_mode=DR)

            # DMAs in consumption order. Measured: no packet lands before
            # ~8.7us and the 2MB x2 shard completes only at ~17us
            # (150-380GB/s ramp), so the head must consume x2 at below
            # delivery rate: x1 tiles 0-3, then the 16 [k-pair, j-block]
            # 128KB pieces of x2 in the j-major order the head panel
            # consumes them, then the rest of x1 batched 8 tiles per DMA
            # (transposed source AP puts the partition dim first to match
            # the SBUF iteration order).
            # interleave the head x1 tiles with j-block 0's pieces in exact
            # consumption order: mi=0 needs x1t[0]+k0j0 first; x1t[1..3]
            # queued ahead of x2 delayed k0j0 past the warm-up drain and
            # cost a 2.2us gap + HAM re-throttle.
            nc.sync.dma_start(out=x1t_s[:, 0], in_=x1t[0])
            nc.sync.dma_start(out=x2t_s[:, 0:2, 0:JBLK],
                              in_=x2t[:, 0:2, 0:JBLK])
            for t in range(1, KP):
                nc.sync.dma_start(out=x1t_s[:, t], in_=x1t[t])
            for t in range(1, KP):
                ks = slice(2 * t, 2 * t + 2)
                nc.sync.dma_start(out=x2t_s[:, ks, 0:JBLK],
                                  in_=x2t[:, ks, 0:JBLK])
            for jb in range(1, JB):
                js = slice(jb * JBLK, (jb + 1) * JBLK)
                for t in range(KP):
                    ks = slice(2 * t, 2 * t + 2)
                    nc.sync.dma_start(out=x2t_s[:, ks, js], in_=x2t[:, ks, js])
            for g in range(HEAD_M, M_TILES, 8):
                hi = min(g + 8, M_TILES)
                nc.sync.dma_start(out=x1t_s[:, g:hi],
                                  in_=x1t[g:hi].transpose([1, 0, 2, 3]))

            # Head panel: m-tiles 0-3 j-block-outer, one PSUM bank per
            # m-tile per j-block. A single m-tile would read x2 at
            # ~590GB/s and starve (run7: 2.4us gap -> HAM re-throttle,
            # +2us); four m-tiles per piece consume at ~148GB/s, under
            # the measured delivery ramp, so the head runs gap-free while
            # x2 streams in.
            for jb in range(JB):
                js = slice(jb * JBLK, (jb + 1) * JBLK)
                pssp = [psum.tile([P, JBLK], F32, tag="ps", name="ps")
                        for _ in range(HEAD_M)]
                # t-outer / m-inner: each 128KB x2 piece is consumed over
                # HEAD_M matmuls (~864ns), matching its arrival spacing --
                # m-outer burned all 4 pieces of a j-block in the first
                # m-tile's 0.86us and still gapped ~1us at jb0.
                for t in range(KP):
                    ks = slice(2 * t, 2 * t + 2)
                    for mi in range(HEAD_M):
                        nc.tensor.matmul(
                            pssp[mi][:], x1t_s[:, mi, ks, :], x2t_s[:, ks, js],
                            start=(t == 0), stop=(t == KP - 1), perf_mode=DR,
                        )
                for mi in range(HEAD_M):
                    nc.vector.tensor_reduce(
                        cmax[:, mi, jb * BPP:(jb + 1) * BPP],
                        pssp[mi][:].rearrange("p (b w) -> p b w", b=BPP),
                        axis=AX.X, op=ALU.max,
                    )

            for m in range(HEAD_M, M_TILES):
                pss = [psum.tile([P, JBLK], F32, tag="ps", name="ps")
                       for _ in range(JB)]
                for t in range(KP):
                    ks = slice(2 * t, 2 * t + 2)
                    for jb in range(JB):
                        js = slice(jb * JBLK, (jb + 1) * JBLK)
                        nc.tensor.matmul(
                            pss[jb][:], x1t_s[:, m, ks, :], x2t_s[:, ks, js],
                            start=(t == 0), stop=(t == KP - 1), perf_mode=DR,
                        )
                for jb in range(JB):
                    nc.vector.tensor_reduce(
                        cmax[:, m, jb * BPP:(jb + 1) * BPP],
                        pss[jb][:].rearrange("p (b w) -> p b w", b=BPP),
                        axis=AX.X, op=ALU.max,
                    )
                # drain to HBM every 8 m-tiles; finer cadence at the end so
                # the post-last-matmul DMA is small.
                if m < 56 and (m + 1) % 8 == 0:
                    nc.sync.dma_start(
                        out=out[:, m - 7:m + 1, :], in_=cmax[:, m - 7:m + 1, :]
                    )
                elif m >= 56 and (m + 1) % 2 == 0:
                    nc.sync.dma_start(
                        out=out[:, m - 1:m + 1, :], in_=cmax[:, m - 1:m + 1, :]
                    )

    nc.finalize()
    return nc


_cache = {}


def _get_nc():
    if "nc" not in _cache:
        _cache["nc"] = build_nc()
    return _cache["nc"]


def _normalize(x1, x2):
    x1 = np.ascontiguousarray(x1, dtype=np.float32)
    x2 = np.ascontiguousarray(x2, dtype=np.float32)
    eps = np.float32(1e-8)
    n1 = np.maximum(np.sqrt(np.einsum("ij,ij->i", x1, x1)), eps)
    n2 = np.maximum(np.sqrt(np.einsum("ij,ij->i", x2, x2)), eps)
    return x1 / n1[:, None], x2 / n2[:, None]


def _prep_inputs(x1n, x2n):
    """fp8-quantize (scaled), transpose + tile + shard."""
    q1 = (x1n * SCALE).astype(ml_dtypes.float8_e4m3)
    q2 = (x2n * SCALE).astype(ml_dtypes.float8_e4m3)

    # [N1, D] -> [m, dp, k, q] with x1t[m, dp, k, q] = q1[m*128+q, k*128+dp]
    x1t = np.ascontiguousarray(
        q1.reshape(M_TILES, P, K_TILES, P).transpose(0, 3, 2, 1)
    )
    in_maps = []
    for c in range(NCORES):
        sl = slice(c * JS, (c + 1) * JS)
        # [JS, D] -> [dp, k, j] with x2t[dp, k, j] = q2[sl][j, k*128+dp]
        x2t = np.ascontiguousarray(
            q2[sl].T.reshape(K_TILES, P, JS).transpose(1, 0, 2)
        )
        in_maps.append({"x1t": x1t, "x2t": x2t})
    return in_maps


def _host_rescore(bm, x1n, x2n):
    """bm: [8192, NCORES*NBLK] screen block maxes (sim units).
    Rescore every block within DELTA of each query's max in bf16."""
    b16 = lambda a: np.ascontiguousarray(a.astype(ml_dtypes.bfloat16)
                                          .astype(np.float32))
    x1b = b16(x1n)
    x2b = b16(x2n)
    gmax = bm.max(axis=1, keepdims=True)
    sel = bm >= gmax - np.float32(DELTA)
    out = np.full(N1, -np.inf, dtype=np.float32)
    nb_total = NCORES * NBLK
    for b in range(nb_total):
        qs = np.nonzero(sel[:, b])[0]
        if len(qs) == 0:
            continue
        blk = x1b[qs] @ x2b[b * BW:(b + 1) * BW].T
        np.maximum.at(out, qs, blk.max(axis=1))
    return out


def run(x1, x2, trace=False):
    nc = _get_nc()
    x1n, x2n = _normalize(np.asarray(x1), np.asarray(x2))
    in_maps = _prep_inputs(x1n, x2n)
    res = run_bass_kernel_spmd(nc, in_maps, core_ids=list(range(NCORES)),
                               trace=trace)
    # device out[q, m, b] = screen blockmax of query m*128+q, local block b
    bm = np.concatenate(
        [np.asarray(res.results[c]["out"]).reshape(P, M_TILES, NBLK)
         .transpose(1, 0, 2).reshape(N1, NBLK) for c in range(NCORES)],
        axis=1,
    ) / (SCALE * SCALE)
    out = _host_rescore(bm, x1n, x2n)
    return np.ascontiguousarray(out.astype(np.float32)), res


def kernel(x1, x2):
    out, _ = run(np.asarray(x1), np.asarray(x2), trace=False)
    return out
